# revision 7
# baseline (speedup 1.0000x reference)
"""Self-contained TRN2 Bass kernel for causal self-attention (B=2,T=2048,D=1024,H=16).

kernel(**inputs) takes the full unsharded inputs and returns the full output.
Sharding: 8 NeuronCores; core c -> batch b=c//4, head-group g=c%4 (4 heads).
Each core runs projections + RoPE + causal flash-style attention (transposed
scores, deferred softmax normalization) + a partial output projection; the
host sums the 4 per-batch partials and adds the output bias.

v2: softmax reciprocal on DVE (reciprocal_approx_fast on a [16,128] reshape
of the row sums) so ACT only ever runs Exp (one table load); AV staging and
V staging on DVE; V projection split into 4 t-groups interleaved with the
flash loops to keep the PE warm while ACT drains exps; bf16 at/wo; per-slab
weight DMAs + split DMA queues to cut the startup stall.
"""

import math
from contextlib import ExitStack

import numpy as np

import concourse.bass as bass
import concourse.tile as tile
from concourse import bacc, mybir

F32 = mybir.dt.float32
BF16 = mybir.dt.bfloat16

B, T, D, H, HD = 2, 2048, 1024, 16, 64
P = 128
KT = D // P            # 8 k-slabs for projections
NT = T // P            # 16 t/k tiles
QS = 512               # q-slab width for attention
NQS = T // QS          # 4 q-slabs
HPG = 4                # heads per core


def build_nc(num_devices=8):
    nc = bacc.Bacc("TRN2", target_bir_lowering=False, debug=False,
                   num_devices=num_devices)
    ext = dict(kind="ExternalInput")
    xT = nc.dram_tensor("xT", [D, T], BF16, **ext).ap()
    wq = nc.dram_tensor("wq", [D, 2 * P], BF16, **ext).ap()
    wk = nc.dram_tensor("wk", [D, 2 * P], BF16, **ext).ap()
    wv = nc.dram_tensor("wv", [D, 2 * P], BF16, **ext).ap()
    wo = nc.dram_tensor("wo", [2 * P, D], BF16, **ext).ap()
    csc = nc.dram_tensor("csc", [P, T], BF16, **ext).ap()
    ssc = nc.dram_tensor("ssc", [P, T], BF16, **ext).ap()
    bq2 = nc.dram_tensor("bq2", [P, 2], F32, **ext).ap()
    bk2 = nc.dram_tensor("bk2", [P, 2], F32, **ext).ap()
    bvr = nc.dram_tensor("bvr", [1, 2 * P], BF16, **ext).ap()
    m0 = nc.dram_tensor("m0", [P, P], BF16, **ext).ap()
    ones_in = nc.dram_tensor("ones_in", [P, P], BF16, **ext).ap()
    out = nc.dram_tensor("out", [T, D], F32, kind="ExternalOutput").ap()

    with tile.TileContext(nc) as tc:
        _body(tc, xT, wq, wk, wv, wo, csc, ssc, bq2, bk2, bvr, m0,
              ones_in, out)
    nc.compile()
    return nc


def _body(tc, xT, wq, wk, wv, wo, csc, ssc, bq2, bk2, bvr, m0,
          ones_in, out):
    nc = tc.nc
    Copy = mybir.ActivationFunctionType.Copy
    Exp = mybir.ActivationFunctionType.Exp
    Ident = mybir.ActivationFunctionType.Identity

    with ExitStack() as outer:
        consts = outer.enter_context(tc.tile_pool(name="consts", bufs=1))
        wpool = outer.enter_context(tc.tile_pool(name="w", bufs=1))
        xtp = outer.enter_context(tc.tile_pool(name="xt", bufs=1))
        qk = outer.enter_context(tc.tile_pool(name="qk", bufs=1))
        vp = outer.enter_context(tc.tile_pool(name="v", bufs=1))
        atp = outer.enter_context(tc.tile_pool(name="at", bufs=1))
        cscp = outer.enter_context(tc.tile_pool(name="cs", bufs=1))

        # DMA queues: gpsimd carries weights/consts + xts 4-7, sync carries
        # xts 0-3 (slab 0 column-chunked so the first matmul starts early),
        # vector carries csc/ssc (rope waits on them anyway). The scalar
        # queue issues nothing: ACT time is reserved for staging + exp.
        w_q = wpool.tile([P, KT, 2 * P], BF16, tag="wq")
        w_k = wpool.tile([P, KT, 2 * P], BF16, tag="wk")
        wv_s = wpool.tile([P, KT, 2 * P], BF16, tag="wv")
        wqr = wq.rearrange("(ko ki) m -> ki ko m", ki=P)
        nc.gpsimd.dma_start(w_q[:, 0, :], wqr[:, 0, :])
        nc.gpsimd.dma_start(w_q[:, 1:KT, :], wqr[:, 1:KT, :])

        xts = []
        xTr = xT.rearrange("(ko ki) t -> ki ko t", ki=P)
        for kt in range(KT):
            xc = xtp.tile([P, T], BF16, tag=f"xt{kt}")
            xts.append(xc)
        for n in range(4):
            cols = slice(n * 512, (n + 1) * 512)
            nc.sync.dma_start(xts[0][:, cols], xTr[:, 0, cols])
        for kt in (1, 2, 3):
            nc.sync.dma_start(xts[kt][:], xTr[:, kt, :])
        nc.gpsimd.dma_start(xts[4][:], xTr[:, 4, :])
        nc.gpsimd.dma_start(xts[5][:], xTr[:, 5, :])

        bq_s = consts.tile([P, 2], F32, tag="bq")
        bk_s = consts.tile([P, 2], F32, tag="bk")
        nc.gpsimd.dma_start(bq_s[:], bq2)
        nc.gpsimd.dma_start(bk_s[:], bk2)
        wkr = wk.rearrange("(ko ki) m -> ki ko m", ki=P)
        nc.gpsimd.dma_start(w_k[:], wkr)
        nc.gpsimd.dma_start(xts[6][:], xTr[:, 6, :])
        nc.gpsimd.dma_start(xts[7][:], xTr[:, 7, :])
        wvr = wv.rearrange("(ko ki) m -> ki ko m", ki=P)
        nc.gpsimd.dma_start(wv_s[:], wvr)
        bv_s = consts.tile([1, 2 * P], BF16, tag="bv")
        nc.gpsimd.dma_start(bv_s[:], bvr)
        m0_s = consts.tile([P, P], BF16, tag="m0")
        nc.gpsimd.dma_start(m0_s[:], m0)
        ones_s = consts.tile([1, P], BF16, tag="ones")
        nc.gpsimd.dma_start(ones_s[:], ones_in[0:1, :])
        wop = outer.enter_context(tc.tile_pool(name="wo", bufs=1))
        wo_s = wop.tile([P, 2, D], BF16)
        nc.gpsimd.dma_start(
            wo_s[:], wo.rearrange("(ko ki) m -> ki ko m", ki=P))

        # V sbuf store (per kt-tile, per head, HD cols + ones col for sums)
        v_s = vp.tile([P, NT, HPG, HD + 1], BF16)
        nc.vector.memset(v_s[:, :, :, HD:HD + 1], 1.0)
        csc_s = cscp.tile([P, T], BF16, tag="c")
        nc.sync.dma_start(csc_s[:], csc)
        ssc_s = cscp.tile([P, T], BF16, tag="s")
        nc.sync.dma_start(ssc_s[:], ssc)

        at0 = atp.tile([P, T], BF16, tag="at0")
        at1 = atp.tile([P, T], BF16, tag="at1")
        at_tiles = (at0, at1)

        qc, kc = [], []

        def emit_vgroup(pool, tag, g):
            # V projection for t-tiles 4g..4g+3 -> v_s (+bias via ones row)
            tps = (2 * g, 2 * g + 1)
            vss = {}
            for tp in tps:
                vss[tp] = pool.tile([P, 2, 2 * P], F32, tag=tag,
                                    name=f"vps{tp}")
            for kt in range(KT):
                for tp in tps:
                    for half in range(2):
                        t = 2 * tp + half
                        nc.tensor.matmul(
                            vss[tp][:, half, :],
                            xts[kt][:, t * P:(t + 1) * P],
                            wv_s[:, kt, :],
                            start=(kt == 0 and half == 0), stop=False,
                        )
            for tp in tps:
                for half in range(2):
                    nc.tensor.matmul(
                        vss[tp][:, half, :], ones_s[:], bv_s[:],
                        start=False, stop=(half == 1),
                    )
                nc.vector.tensor_copy(
                    v_s[:, 2 * tp:2 * tp + 2, :, 0:HD],
                    vss[tp].rearrange("p c (h d) -> p c h d", h=HPG),
                )

        # ============ phase A: Q/K projections + RoPE, V groups 0-2 ========
        with ExitStack() as pha:
            rtmp = pha.enter_context(tc.tile_pool(name="rtmp", bufs=4))
            ps_proj = pha.enter_context(
                tc.tile_pool(name="psp", bufs=8, space="PSUM"))

            raw = {}
            for name, w_s, b_s in (("q", w_q, bq_s), ("k", w_k, bk_s)):
                pss = []
                for _i in range(8):
                    pst = ps_proj.tile([P, 512], F32, tag="ps", name=f"ps{_i}")
                    pss.append(pst)
                for kt in range(KT):
                    for m in range(2):
                        for n in range(T // 512):
                            nc.tensor.matmul(
                                pss[m * 4 + n][:],
                                w_s[:, kt, m * P:(m + 1) * P],
                                xts[kt][:, n * 512:(n + 1) * 512],
                                start=(kt == 0), stop=(kt == KT - 1),
                            )
                for m in range(2):
                    rt = qk.tile([P, T], BF16, tag=f"raw{name}{m}")
                    raw[(name, m)] = rt
                    for n in range(T // 512):
                        nc.scalar.activation(
                            out=rt[:, n * 512:(n + 1) * 512],
                            in_=pss[m * 4 + n][:],
                            func=Ident, bias=b_s[:, m:m + 1], scale=1.0,
                        )

                # RoPE + merge right after each projection's staging.
                # Layout per merged tile j: heads (2j, 2j+1); head h occupies
                # partitions 64*(h%2)..+63 as [x1(32); x2(32)].
                x1, x2 = raw[(name, 0)], raw[(name, 1)]
                tgts = []
                for j in range(2):
                    tgt = qk.tile([P, T], BF16, tag=f"c{name}{j}")
                    tgts.append(tgt)
                (qc if name == "q" else kc).extend(tgts)
                # K processed in column halves so qs=0 scores can start
                # before the full-T chain finishes; Q in one pass.
                nch = 1 if name == "q" else 2
                cw = T // nch
                for ch in range(nch):
                    cols = slice(ch * cw, (ch + 1) * cw)
                    t1 = rtmp.tile([P, cw], BF16, tag="rt", name="t1")
                    nc.vector.tensor_mul(t1[:], x1[:, cols], csc_s[:, cols])
                    t2 = rtmp.tile([P, cw], BF16, tag="rt", name="t2")
                    nc.vector.tensor_mul(t2[:], x2[:, cols], ssc_s[:, cols])
                    t3 = rtmp.tile([P, cw], BF16, tag="rt", name="t3")
                    nc.vector.tensor_mul(t3[:], x1[:, cols], ssc_s[:, cols])
                    t4 = rtmp.tile([P, cw], BF16, tag="rt", name="t4")
                    nc.vector.tensor_mul(t4[:], x2[:, cols], csc_s[:, cols])
                    y1 = qk.tile([P, cw], BF16, tag=f"y{name}0", name="y1")
                    nc.vector.tensor_sub(y1[:], t1[:], t2[:])
                    y2 = qk.tile([P, cw], BF16, tag=f"y{name}1", name="y2")
                    nc.vector.tensor_add(y2[:], t3[:], t4[:])
                    # merge halves into head-interleaved tiles (DVE copies;
                    # ScalarE copies run at 1x and stall the ACT queue)
                    for j in range(2):
                        tgt = tgts[j]
                        for i in range(2):
                            h = 2 * j + i
                            hs = slice(32 * h, 32 * h + 32)
                            nc.vector.tensor_copy(
                                tgt[64 * i:64 * i + 32, cols], y1[hs, :])
                            nc.vector.tensor_copy(
                                tgt[64 * i + 32:64 * i + 64, cols], y2[hs, :])

            emit_vgroup(ps_proj, "ps", 0)

        # ================= phase B: attention =================
        with ExitStack() as phb:
            expp = phb.enter_context(tc.tile_pool(name="exp", bufs=6))
            avsp = phb.enter_context(tc.tile_pool(name="avs", bufs=2))
            rrp = phb.enter_context(tc.tile_pool(name="rr", bufs=2))
            s16p = phb.enter_context(tc.tile_pool(name="s16", bufs=2))
            ps_sc = phb.enter_context(
                tc.tile_pool(name="pssc", bufs=2, space="PSUM"))
            ps_av = phb.enter_context(
                tc.tile_pool(name="psav", bufs=1, space="PSUM"))
            drp = phb.enter_context(
                tc.tile_pool(name="dr", bufs=2, space="DRAM"))
            outb = phb.enter_context(tc.tile_pool(name="outb", bufs=3))

            staged = {}

            def emit_rchain(qs):
                # avs65 <- AV psum (incl. sums row 64); r = 1/sums via DVE
                # reciprocal_approx_fast on a [16,128] reshape (DRAM hop),
                # broadcast back over the HD partitions.
                avs, _ = staged[qs]
                d_sums = drp.tile([HPG * QS], F32, tag="ds", name="d_sums")
                nc.sync.dma_start(d_sums[None, :],
                                  avs[HD:HD + 1, :, :].rearrange(
                                      "o h q -> o (h q)"))
                s16 = s16p.tile([16, P], F32, tag="s16", name="s16")
                nc.sync.dma_start(s16[:],
                                  d_sums.rearrange("(p c) -> p c", p=16))
                r16 = s16p.tile([16, P], F32, tag="r16", name="r16")
                nc.vector.reciprocal_approx_fast(out=r16[:], in_=s16[:])
                d_r = drp.tile([HPG * QS], F32, tag="dr", name="d_r")
                nc.sync.dma_start(d_r.rearrange("(p c) -> p c", p=16), r16[:])
                rr = rrp.tile([HD, HPG, QS], F32, tag="rr", name="rr")
                nc.sync.dma_start(
                    rr[:], d_r[None, :].broadcast_to([HD, HPG * QS])
                    .rearrange("p (h q) -> p h q", h=HPG))
                staged[qs] = (avs, rr)

            def emit_normalize(qs, fine=False):
                avs, rr = staged.pop(qs)
                if not fine:
                    for tg in range(2):
                        for i in range(2):
                            h = 2 * tg + i
                            nc.vector.tensor_mul(
                                at_tiles[tg][i * HD:(i + 1) * HD,
                                             qs * QS:(qs + 1) * QS],
                                avs[0:HD, h, :], rr[:, h, :],
                            )
                else:
                    # per-qt columns so outproj can chase the muls
                    for qt in range(4 * qs, 4 * qs + 4):
                        c0 = qt * P - qs * QS
                        for tg in range(2):
                            for i in range(2):
                                h = 2 * tg + i
                                nc.vector.tensor_mul(
                                    at_tiles[tg][i * HD:(i + 1) * HD,
                                                 qt * P:(qt + 1) * P],
                                    avs[0:HD, h, c0:c0 + P],
                                    rr[:, h, c0:c0 + P],
                                )
                        emit_outproj_qt(qt)

            def emit_outproj_qt(qt):
                ps = ps_sc.tile([P, 2, QS], F32, tag="sc", name="ops")
                ob = outb.tile([P, D], F32, tag="ob", name="ob")
                for nb in range(2):
                    for ktg in range(2):
                        nc.tensor.matmul(
                            ps[:, nb, :],
                            at_tiles[ktg][:, qt * P:(qt + 1) * P],
                            wo_s[:, ktg, nb * 512:(nb + 1) * 512],
                            start=(ktg == 0), stop=(ktg == 1),
                        )
                    nc.vector.tensor_copy(
                        out=ob[:, nb * 512:(nb + 1) * 512],
                        in_=ps[:, nb, :])
                nc.gpsimd.dma_start(out[qt * P:(qt + 1) * P, :], ob[:])

            def emit_outproj(qs):
                for qt in range(4 * qs, 4 * qs + 4):
                    emit_outproj_qt(qt)

            # Descending q-slab order: the biggest exp workload starts first
            # (keeps ACT saturated while later PE work drains), the kernel
            # ends on the smallest slab, and out-DMAs spread evenly. V
            # groups 1-3 are interleaved into the qs=3 kt-loop right before
            # the AV matmuls that need them, filling PE while ACT chews the
            # early exp backlog.
            QORDER = [3, 2, 1, 0]
            for qidx, qs in enumerate(QORDER):
                av = ps_av.tile([HD + 1, HPG, QS], F32, tag="av", name="av")
                n_kt = 4 * qs + 4
                for kt in range(n_kt):
                    if qidx == 0 and kt % 4 == 0 and kt > 0:
                        emit_vgroup(ps_sc, "sc", kt // 4)
                    qoff = max(0, kt * P - qs * QS)
                    q0 = qs * QS + qoff
                    qext = QS - qoff
                    diag = kt * P >= qs * QS
                    for pair in range(2):
                        sc = ps_sc.tile([P, 2, QS], F32, tag="sc", name="sc")
                        for i in range(2):
                            nc.tensor.matmul(
                                sc[:, i, qoff:QS],
                                kc[pair][64 * i:64 * i + 64,
                                         kt * P:(kt + 1) * P],
                                qc[pair][64 * i:64 * i + 64, q0:q0 + qext],
                                start=True, stop=True,
                                tile_position=(64 * i, 0),
                            )
                        ex = expp.tile([P, 2, QS], BF16, tag="ex", name="ex")
                        nc.scalar.activation(
                            out=ex[:, :, qoff:QS], in_=sc[:, :, qoff:QS],
                            func=Exp, scale=1.0,
                        )
                        if diag:
                            # diagonal k-tile: zero strictly-upper corner
                            nc.vector.tensor_mul(
                                ex[:, :, qoff:qoff + P],
                                ex[:, :, qoff:qoff + P],
                                m0_s[:, None, :].broadcast_to([P, 2, P]),
                            )
                        for i in range(2):
                            h = 2 * pair + i
                            nc.tensor.matmul(
                                av[:, h, qoff:QS],
                                v_s[:, kt, h, :],
                                ex[:, i, qoff:QS],
                                start=(kt == 0), stop=(kt == n_kt - 1),
                            )
                # stage AV psum (+ sums row) to SBUF, then the r chain
                avs = avsp.tile([HD + 1, HPG, QS], F32, tag="avs", name="avs")
                nc.vector.tensor_copy(avs[:], av[:])
                staged[qs] = (avs, None)
                emit_rchain(qs)

                if qidx > 0:
                    emit_normalize(QORDER[qidx - 1])
                    emit_outproj(QORDER[qidx - 1])

            emit_normalize(QORDER[-1], fine=True)


# ---------------- host-side prep ----------------

def _perm(g):
    perm = []
    for half in range(2):
        for hh in range(HPG):
            for i in range(32):
                perm.append(256 * g + 64 * hh + 2 * i + half)
    return np.array(perm)


def host_inputs(inputs, c):
    b, g = c // 4, c % 4
    x, cos, sin = inputs["x"], inputs["cos"], inputs["sin"]
    Wq, bq, Wk, bk = inputs["Wq"], inputs["bq"], inputs["Wk"], inputs["bk"]
    Wv, bv, Wo = inputs["Wv"], inputs["bv"], inputs["Wo"]
    perm = _perm(g)
    s = math.sqrt(1.0 / math.sqrt(HD))
    cosT = np.ascontiguousarray(cos[0, 0].T) * s    # [32, T]
    sinT = np.ascontiguousarray(sin[0, 0].T) * s
    f32 = np.float32
    import ml_dtypes
    bf16 = ml_dtypes.bfloat16
    return {
        "xT": np.ascontiguousarray(x[b].T).astype(bf16),
        "wq": np.ascontiguousarray(Wq[perm, :].T).astype(bf16),
        "wk": np.ascontiguousarray(Wk[perm, :].T).astype(bf16),
        "wv": np.ascontiguousarray(Wv[256 * g:256 * (g + 1), :].T).astype(bf16),
        "wo": np.ascontiguousarray(Wo[:, 256 * g:256 * (g + 1)].T).astype(bf16),
        "csc": np.ascontiguousarray(np.tile(cosT, (4, 1))).astype(bf16),
        "ssc": np.ascontiguousarray(np.tile(sinT, (4, 1))).astype(bf16),
        "bq2": np.ascontiguousarray(bq[perm].reshape(2, P).T).astype(f32),
        "bk2": np.ascontiguousarray(bk[perm].reshape(2, P).T).astype(f32),
        "bvr": np.ascontiguousarray(
            bv[256 * g:256 * (g + 1)].reshape(1, 2 * P)).astype(bf16),
        "m0": np.ascontiguousarray(
            (np.arange(P)[None, :] >= np.arange(P)[:, None])).astype(bf16),
        "ones_in": np.ones((P, P), bf16),
    }


def host_gather(results, bo):
    out = np.zeros((B, T, D), np.float32)
    for c in range(8):
        out[c // 4] += results[c]["out"]
    out += bo[None, None, :]
    return out


_NC_CACHE = {}


def _get_nc():
    if "nc" not in _NC_CACHE:
        _NC_CACHE["nc"] = build_nc(num_devices=8)
    return _NC_CACHE["nc"]


def kernel(**inputs):
    inputs = {k: np.asarray(v) for k, v in inputs.items()}
    nc = _get_nc()
    from concourse.bass_utils import run_bass_kernel_spmd
    in_maps = [host_inputs(inputs, c) for c in range(8)]
    res = run_bass_kernel_spmd(nc, in_maps, core_ids=list(range(8)))
    return host_gather(res.results, inputs["bo"].astype(np.float32))


# revision 15
# speedup vs baseline: 1.0217x; 1.0217x over previous
"""Self-contained TRN2 Bass kernel for causal self-attention (B=2,T=2048,D=1024,H=16).

kernel(**inputs) takes the full unsharded inputs and returns the full output.
Sharding: 8 NeuronCores; core c -> batch b=c//4, head-group g=c%4 (4 heads).
Each core runs projections + RoPE + causal flash-style attention (transposed
scores, deferred softmax normalization) + a partial output projection; the
host sums the 4 per-batch partials and adds the output bias.

v2: softmax reciprocal on DVE (reciprocal_approx_fast on a [16,128] reshape
of the row sums) so ACT only ever runs Exp (one table load); AV staging and
V staging on DVE; V projection split into 4 t-groups interleaved with the
flash loops to keep the PE warm while ACT drains exps; bf16 at/wo; per-slab
weight DMAs + split DMA queues to cut the startup stall.
"""

import math
from contextlib import ExitStack

import numpy as np

import concourse.bass as bass
import concourse.tile as tile
from concourse import bacc, mybir

F32 = mybir.dt.float32
BF16 = mybir.dt.bfloat16

B, T, D, H, HD = 2, 2048, 1024, 16, 64
P = 128
KT = D // P            # 8 k-slabs for projections
NT = T // P            # 16 t/k tiles
QS = 512               # q-slab width for attention
NQS = T // QS          # 4 q-slabs
HPG = 4                # heads per core


def build_nc(num_devices=8):
    nc = bacc.Bacc("TRN2", target_bir_lowering=False, debug=False,
                   num_devices=num_devices)
    ext = dict(kind="ExternalInput")
    xT = nc.dram_tensor("xT", [D, T], BF16, **ext).ap()
    wq = nc.dram_tensor("wq", [D, 2 * P], BF16, **ext).ap()
    wk = nc.dram_tensor("wk", [D, 2 * P], BF16, **ext).ap()
    wv = nc.dram_tensor("wv", [D, 2 * P], BF16, **ext).ap()
    wo = nc.dram_tensor("wo", [2 * P, D], BF16, **ext).ap()
    csc = nc.dram_tensor("csc", [P, T], BF16, **ext).ap()
    ssc = nc.dram_tensor("ssc", [P, T], BF16, **ext).ap()
    bq2 = nc.dram_tensor("bq2", [P, 2], F32, **ext).ap()
    bk2 = nc.dram_tensor("bk2", [P, 2], F32, **ext).ap()
    bvr = nc.dram_tensor("bvr", [1, 2 * P], BF16, **ext).ap()
    m0 = nc.dram_tensor("m0", [P, P], BF16, **ext).ap()
    ones_in = nc.dram_tensor("ones_in", [P, P], BF16, **ext).ap()
    out = nc.dram_tensor("out", [T, D], F32, kind="ExternalOutput").ap()

    with tile.TileContext(nc) as tc:
        _body(tc, xT, wq, wk, wv, wo, csc, ssc, bq2, bk2, bvr, m0,
              ones_in, out)
    nc.compile()
    return nc


def _body(tc, xT, wq, wk, wv, wo, csc, ssc, bq2, bk2, bvr, m0,
          ones_in, out):
    nc = tc.nc
    Copy = mybir.ActivationFunctionType.Copy
    Exp = mybir.ActivationFunctionType.Exp
    Ident = mybir.ActivationFunctionType.Identity

    with ExitStack() as outer:
        consts = outer.enter_context(tc.tile_pool(name="consts", bufs=1))
        wpool = outer.enter_context(tc.tile_pool(name="w", bufs=1))
        xtp = outer.enter_context(tc.tile_pool(name="xt", bufs=1))
        qk = outer.enter_context(tc.tile_pool(name="qk", bufs=1))
        vp = outer.enter_context(tc.tile_pool(name="v", bufs=1))
        atp = outer.enter_context(tc.tile_pool(name="at", bufs=1))
        cscp = outer.enter_context(tc.tile_pool(name="cs", bufs=1))

        # DMA queues: gpsimd carries weights/consts + xts 4-7, sync carries
        # xts 0-3 (slab 0 column-chunked so the first matmul starts early),
        # vector carries csc/ssc (rope waits on them anyway). The scalar
        # queue issues nothing: ACT time is reserved for staging + exp.
        w_q = wpool.tile([P, KT, 2 * P], BF16, tag="wq")
        w_k = wpool.tile([P, KT, 2 * P], BF16, tag="wk")
        wv_s = wpool.tile([P, KT, 2 * P], BF16, tag="wv")
        wkr = wk.rearrange("(ko ki) m -> ki ko m", ki=P)
        nc.gpsimd.dma_start(w_k[:, 0, :], wkr[:, 0, :])
        nc.gpsimd.dma_start(w_k[:, 1:KT, :], wkr[:, 1:KT, :])

        xts = []
        xTr = xT.rearrange("(ko ki) t -> ki ko t", ki=P)
        for kt in range(KT):
            xc = xtp.tile([P, T], BF16, tag=f"xt{kt}")
            xts.append(xc)
        for n in range(4):
            cols = slice(n * 512, (n + 1) * 512)
            nc.sync.dma_start(xts[0][:, cols], xTr[:, 0, cols])
        for kt in (1, 2, 3):
            nc.sync.dma_start(xts[kt][:], xTr[:, kt, :])
        nc.gpsimd.dma_start(xts[4][:], xTr[:, 4, :])
        nc.gpsimd.dma_start(xts[5][:], xTr[:, 5, :])

        bq_s = consts.tile([P, 2], F32, tag="bq")
        bk_s = consts.tile([P, 2], F32, tag="bk")
        nc.gpsimd.dma_start(bk_s[:], bk2)
        nc.gpsimd.dma_start(bq_s[:], bq2)
        wqr = wq.rearrange("(ko ki) m -> ki ko m", ki=P)
        nc.gpsimd.dma_start(w_q[:], wqr)
        nc.gpsimd.dma_start(xts[6][:], xTr[:, 6, :])
        nc.gpsimd.dma_start(xts[7][:], xTr[:, 7, :])
        wvr = wv.rearrange("(ko ki) m -> ki ko m", ki=P)
        nc.gpsimd.dma_start(wv_s[:], wvr)
        bv_s = consts.tile([1, 2 * P], BF16, tag="bv")
        nc.gpsimd.dma_start(bv_s[:], bvr)
        m0_s = consts.tile([P, P], BF16, tag="m0")
        nc.gpsimd.dma_start(m0_s[:], m0)
        ones_s = consts.tile([1, P], BF16, tag="ones")
        nc.gpsimd.dma_start(ones_s[:], ones_in[0:1, :])
        wop = outer.enter_context(tc.tile_pool(name="wo", bufs=1))
        wo_s = wop.tile([P, 2, D], BF16)
        nc.gpsimd.dma_start(
            wo_s[:], wo.rearrange("(ko ki) m -> ki ko m", ki=P))

        # V sbuf store (per kt-tile, per head, HD cols + ones col for sums)
        v_s = vp.tile([P, NT, HPG, HD + 1], BF16)
        nc.vector.memset(v_s[:, :, :, HD:HD + 1], 1.0)
        csc_s = cscp.tile([P, T], BF16, tag="c")
        nc.sync.dma_start(csc_s[:], csc)
        ssc_s = cscp.tile([P, T], BF16, tag="s")
        nc.sync.dma_start(ssc_s[:], ssc)

        at0 = atp.tile([P, T], BF16, tag="at0")
        at1 = atp.tile([P, T], BF16, tag="at1")
        at_tiles = (at0, at1)

        qc, kc = [], []

        def emit_vgroup(pool, tag, g):
            # V projection for t-tiles 4g..4g+3 -> v_s (+bias via ones row)
            tps = (2 * g, 2 * g + 1)
            vss = {}
            for tp in tps:
                vss[tp] = pool.tile([P, 2, 2 * P], F32, tag=tag,
                                    name=f"vps{tp}")
            for kt in range(KT):
                for tp in tps:
                    for half in range(2):
                        t = 2 * tp + half
                        nc.tensor.matmul(
                            vss[tp][:, half, :],
                            xts[kt][:, t * P:(t + 1) * P],
                            wv_s[:, kt, :],
                            start=(kt == 0 and half == 0), stop=False,
                        )
            for tp in tps:
                for half in range(2):
                    nc.tensor.matmul(
                        vss[tp][:, half, :], ones_s[:], bv_s[:],
                        start=False, stop=(half == 1),
                    )
                nc.vector.tensor_copy(
                    v_s[:, 2 * tp:2 * tp + 2, :, 0:HD],
                    vss[tp].rearrange("p c (h d) -> p c h d", h=HPG),
                )

        # ============ phase A: Q/K projections + RoPE, V groups 0-2 ========
        with ExitStack() as pha:
            rtmp = pha.enter_context(tc.tile_pool(name="rtmp", bufs=4))
            ps_proj = pha.enter_context(
                tc.tile_pool(name="psp", bufs=8, space="PSUM"))

            raw = {}
            # K first: its stage/rope/merge chain runs on ACT/DVE while the
            # PE grinds the Q projection. K in ascending column halves (the
            # first flash consumes low k first); Q in DESCENDING column
            # quarters (flash(qs=3) reads q columns 1536+ first).
            for name, w_s, b_s, chunks in (
                    ("k", w_k, bk_s, [0, 1]),
                    ("q", w_q, bq_s, [3, 2, 1, 0])):
                nch = len(chunks)
                cw = T // nch
                pss = []
                for _i in range(8):
                    pst = ps_proj.tile([P, 512], F32, tag="ps", name=f"ps{_i}")
                    pss.append(pst)
                for kt in range(KT):
                    for m in range(2):
                        for n in range(T // 512):
                            nc.tensor.matmul(
                                pss[m * 4 + n][:],
                                w_s[:, kt, m * P:(m + 1) * P],
                                xts[kt][:, n * 512:(n + 1) * 512],
                                start=(kt == 0), stop=(kt == KT - 1),
                            )
                for m in range(2):
                    rt = qk.tile([P, T], BF16, tag=f"raw{name}{m}")
                    raw[(name, m)] = rt
                nst = T // 512
                for n in (range(nst) if name == "k" else range(nst - 1, -1, -1)):
                    for m in range(2):
                        nc.scalar.activation(
                            out=raw[(name, m)][:, n * 512:(n + 1) * 512],
                            in_=pss[m * 4 + n][:],
                            func=Ident, bias=b_s[:, m:m + 1], scale=1.0,
                        )

                # RoPE + merge right after each projection's staging.
                # Layout per merged tile j: heads (2j, 2j+1); head h occupies
                # partitions 64*(h%2)..+63 as [x1(32); x2(32)].
                x1, x2 = raw[(name, 0)], raw[(name, 1)]
                tgts = []
                for j in range(2):
                    tgt = qk.tile([P, T], BF16, tag=f"c{name}{j}")
                    tgts.append(tgt)
                (qc if name == "q" else kc).extend(tgts)
                for ch in chunks:
                    cols = slice(ch * cw, (ch + 1) * cw)
                    t1 = rtmp.tile([P, cw], BF16, tag="rt", name="t1")
                    nc.vector.tensor_mul(t1[:], x1[:, cols], csc_s[:, cols])
                    t2 = rtmp.tile([P, cw], BF16, tag="rt", name="t2")
                    nc.vector.tensor_mul(t2[:], x2[:, cols], ssc_s[:, cols])
                    t3 = rtmp.tile([P, cw], BF16, tag="rt", name="t3")
                    nc.vector.tensor_mul(t3[:], x1[:, cols], ssc_s[:, cols])
                    t4 = rtmp.tile([P, cw], BF16, tag="rt", name="t4")
                    nc.vector.tensor_mul(t4[:], x2[:, cols], csc_s[:, cols])
                    y1 = qk.tile([P, cw], BF16, tag=f"y{name}0", name="y1")
                    nc.vector.tensor_sub(y1[:], t1[:], t2[:])
                    y2 = qk.tile([P, cw], BF16, tag=f"y{name}1", name="y2")
                    nc.vector.tensor_add(y2[:], t3[:], t4[:])
                    # merge halves into head-interleaved tiles (DVE copies;
                    # ScalarE copies run at 1x and stall the ACT queue)
                    for j in range(2):
                        tgt = tgts[j]
                        for i in range(2):
                            h = 2 * j + i
                            hs = slice(32 * h, 32 * h + 32)
                            nc.vector.tensor_copy(
                                tgt[64 * i:64 * i + 32, cols], y1[hs, :])
                            nc.vector.tensor_copy(
                                tgt[64 * i + 32:64 * i + 64, cols], y2[hs, :])

            emit_vgroup(ps_proj, "ps", 0)
            emit_vgroup(ps_proj, "ps", 1)

        # ================= phase B: attention =================
        with ExitStack() as phb:
            expp = phb.enter_context(tc.tile_pool(name="exp", bufs=6))
            avsp = phb.enter_context(tc.tile_pool(name="avs", bufs=2))
            rrp = phb.enter_context(tc.tile_pool(name="rr", bufs=2))
            s16p = phb.enter_context(tc.tile_pool(name="s16", bufs=2))
            ps_sc = phb.enter_context(
                tc.tile_pool(name="pssc", bufs=2, space="PSUM"))
            ps_av = phb.enter_context(
                tc.tile_pool(name="psav", bufs=1, space="PSUM"))
            drp = phb.enter_context(
                tc.tile_pool(name="dr", bufs=2, space="DRAM"))
            outb = phb.enter_context(tc.tile_pool(name="outb", bufs=3))

            staged = {}

            def emit_rchain(qs, sums_src):
                # r = 1/sums via DVE reciprocal_approx_fast on a [16,128]
                # reshape (DRAM hop), broadcast back over the HD partitions
                # as bf16.
                avs, _ = staged[qs]
                d_sums = drp.tile([HPG * QS], F32, tag="ds", name="d_sums")
                nc.sync.dma_start(d_sums[None, :],
                                  sums_src.rearrange("o h q -> o (h q)"))
                s16 = s16p.tile([16, P], F32, tag="s16", name="s16")
                nc.sync.dma_start(s16[:],
                                  d_sums.rearrange("(p c) -> p c", p=16))
                r16 = s16p.tile([16, P], F32, tag="r16", name="r16")
                nc.vector.reciprocal_approx_fast(out=r16[:], in_=s16[:])
                r16b = s16p.tile([16, P], BF16, tag="r16b", name="r16b")
                nc.vector.tensor_copy(r16b[:], r16[:])
                d_r = drp.tile([HPG * QS], BF16, tag="dr", name="d_r")
                nc.sync.dma_start(d_r.rearrange("(p c) -> p c", p=16),
                                  r16b[:])
                rr = rrp.tile([HD, HPG, QS], BF16, tag="rr", name="rr")
                nc.sync.dma_start(
                    rr[:], d_r[None, :].broadcast_to([HD, HPG * QS])
                    .rearrange("p (h q) -> p h q", h=HPG))
                staged[qs] = (avs, rr)

            def emit_normalize(qs, fine=False):
                avs, rr = staged.pop(qs)
                if not fine:
                    for tg in range(2):
                        for i in range(2):
                            h = 2 * tg + i
                            nc.vector.tensor_mul(
                                at_tiles[tg][i * HD:(i + 1) * HD,
                                             qs * QS:(qs + 1) * QS],
                                avs[0:HD, h, :], rr[:, h, :],
                            )
                else:
                    # per-qt columns so outproj can chase the muls
                    for qt in range(4 * qs, 4 * qs + 4):
                        c0 = qt * P - qs * QS
                        for tg in range(2):
                            for i in range(2):
                                h = 2 * tg + i
                                nc.vector.tensor_mul(
                                    at_tiles[tg][i * HD:(i + 1) * HD,
                                                 qt * P:(qt + 1) * P],
                                    avs[0:HD, h, c0:c0 + P],
                                    rr[:, h, c0:c0 + P],
                                )
                        emit_outproj_qt(qt, ring=nc.scalar)

            def emit_outproj_qt(qt, ring=None):
                ps = ps_sc.tile([P, 2, QS], F32, tag="sc", name="ops")
                ob = outb.tile([P, D], F32, tag="ob", name="ob")
                for nb in range(2):
                    for ktg in range(2):
                        nc.tensor.matmul(
                            ps[:, nb, :],
                            at_tiles[ktg][:, qt * P:(qt + 1) * P],
                            wo_s[:, ktg, nb * 512:(nb + 1) * 512],
                            start=(ktg == 0), stop=(ktg == 1),
                        )
                    nc.vector.tensor_copy(
                        out=ob[:, nb * 512:(nb + 1) * 512],
                        in_=ps[:, nb, :])
                if ring is None:
                    ring = nc.gpsimd if qt % 2 == 0 else nc.sync
                ring.dma_start(out[qt * P:(qt + 1) * P, :], ob[:])

            def emit_outproj(qs):
                for qt in range(4 * qs, 4 * qs + 4):
                    emit_outproj_qt(qt)

            # Descending q-slab order: the biggest exp workload starts first
            # (keeps ACT saturated while later PE work drains), the kernel
            # ends on the smallest slab, and out-DMAs spread evenly. V
            # groups 1-3 are interleaved into the qs=3 kt-loop right before
            # the AV matmuls that need them, filling PE while ACT chews the
            # early exp backlog.
            QORDER = [3, 2, 1, 0]
            for qidx, qs in enumerate(QORDER):
                av = ps_av.tile([HD + 1, HPG, QS], F32, tag="av", name="av")
                n_kt = 4 * qs + 4
                for kt in range(n_kt):
                    if qidx == 0 and kt in (4, 8):
                        emit_vgroup(ps_sc, "sc", 1 + kt // 4)
                    qoff = max(0, kt * P - qs * QS)
                    q0 = qs * QS + qoff
                    qext = QS - qoff
                    diag = kt * P >= qs * QS
                    for pair in range(2):
                        sc = ps_sc.tile([P, 2, QS], F32, tag="sc", name="sc")
                        for i in range(2):
                            nc.tensor.matmul(
                                sc[:, i, qoff:QS],
                                kc[pair][64 * i:64 * i + 64,
                                         kt * P:(kt + 1) * P],
                                qc[pair][64 * i:64 * i + 64, q0:q0 + qext],
                                start=True, stop=True,
                                tile_position=(64 * i, 0),
                            )
                        ex = expp.tile([P, 2, QS], BF16, tag="ex", name="ex")
                        nc.scalar.activation(
                            out=ex[:, :, qoff:QS], in_=sc[:, :, qoff:QS],
                            func=Exp, scale=1.0,
                        )
                        if diag:
                            # diagonal k-tile: zero strictly-upper corner
                            nc.vector.tensor_mul(
                                ex[:, :, qoff:qoff + P],
                                ex[:, :, qoff:qoff + P],
                                m0_s[:, None, :].broadcast_to([P, 2, P]),
                            )
                        for i in range(2):
                            h = 2 * pair + i
                            nc.tensor.matmul(
                                av[:, h, qoff:QS],
                                v_s[:, kt, h, :],
                                ex[:, i, qoff:QS],
                                start=(kt == 0), stop=(kt == n_kt - 1),
                            )
                # stage AV psum (+ sums row) to SBUF, then the r chain. For
                # the last slab the sums row goes through ACT (idle in the
                # tail) so the r chain doesn't wait on the big DVE copy.
                avs = avsp.tile([HD + 1, HPG, QS], F32, tag="avs", name="avs")
                last = qidx == len(QORDER) - 1
                if last:
                    sums_sb = s16p.tile([1, HPG, QS], F32, tag="sm",
                                        name="sums_sb")
                    nc.scalar.activation(out=sums_sb[:],
                                         in_=av[HD:HD + 1, :, :],
                                         func=Copy, scale=1.0)
                    sums_src = sums_sb[:]
                else:
                    sums_src = avs[HD:HD + 1, :, :]
                nc.vector.tensor_copy(avs[:], av[:])
                staged[qs] = (avs, None)
                emit_rchain(qs, sums_src)

                if qidx > 0:
                    emit_normalize(QORDER[qidx - 1])
                    emit_outproj(QORDER[qidx - 1])

            emit_normalize(QORDER[-1], fine=True)


# ---------------- host-side prep ----------------

def _perm(g):
    perm = []
    for half in range(2):
        for hh in range(HPG):
            for i in range(32):
                perm.append(256 * g + 64 * hh + 2 * i + half)
    return np.array(perm)


def host_inputs(inputs, c):
    b, g = c // 4, c % 4
    x, cos, sin = inputs["x"], inputs["cos"], inputs["sin"]
    Wq, bq, Wk, bk = inputs["Wq"], inputs["bq"], inputs["Wk"], inputs["bk"]
    Wv, bv, Wo = inputs["Wv"], inputs["bv"], inputs["Wo"]
    perm = _perm(g)
    s = math.sqrt(1.0 / math.sqrt(HD))
    cosT = np.ascontiguousarray(cos[0, 0].T) * s    # [32, T]
    sinT = np.ascontiguousarray(sin[0, 0].T) * s
    f32 = np.float32
    import ml_dtypes
    bf16 = ml_dtypes.bfloat16
    return {
        "xT": np.ascontiguousarray(x[b].T).astype(bf16),
        "wq": np.ascontiguousarray(Wq[perm, :].T).astype(bf16),
        "wk": np.ascontiguousarray(Wk[perm, :].T).astype(bf16),
        "wv": np.ascontiguousarray(Wv[256 * g:256 * (g + 1), :].T).astype(bf16),
        "wo": np.ascontiguousarray(Wo[:, 256 * g:256 * (g + 1)].T).astype(bf16),
        "csc": np.ascontiguousarray(np.tile(cosT, (4, 1))).astype(bf16),
        "ssc": np.ascontiguousarray(np.tile(sinT, (4, 1))).astype(bf16),
        "bq2": np.ascontiguousarray(bq[perm].reshape(2, P).T).astype(f32),
        "bk2": np.ascontiguousarray(bk[perm].reshape(2, P).T).astype(f32),
        "bvr": np.ascontiguousarray(
            bv[256 * g:256 * (g + 1)].reshape(1, 2 * P)).astype(bf16),
        "m0": np.ascontiguousarray(
            (np.arange(P)[None, :] >= np.arange(P)[:, None])).astype(bf16),
        "ones_in": np.ones((P, P), bf16),
    }


def host_gather(results, bo):
    out = np.zeros((B, T, D), np.float32)
    for c in range(8):
        out[c // 4] += results[c]["out"]
    out += bo[None, None, :]
    return out


_NC_CACHE = {}


def _get_nc():
    if "nc" not in _NC_CACHE:
        _NC_CACHE["nc"] = build_nc(num_devices=8)
    return _NC_CACHE["nc"]


def kernel(**inputs):
    inputs = {k: np.asarray(v) for k, v in inputs.items()}
    nc = _get_nc()
    from concourse.bass_utils import run_bass_kernel_spmd
    in_maps = [host_inputs(inputs, c) for c in range(8)]
    res = run_bass_kernel_spmd(nc, in_maps, core_ids=list(range(8)))
    return host_gather(res.results, inputs["bo"].astype(np.float32))


# revision 18
# speedup vs baseline: 1.0655x; 1.0428x over previous
"""Self-contained TRN2 Bass kernel for causal self-attention (B=2,T=2048,D=1024,H=16).

kernel(**inputs) takes the full unsharded inputs and returns the full output.
Sharding: 8 NeuronCores; core c -> batch b=c//4, head-group g=c%4 (4 heads).
Each core runs projections + RoPE + causal flash-style attention (transposed
scores, deferred softmax normalization) + a partial output projection; the
host sums the 4 per-batch partials and adds the output bias.

v2: softmax reciprocal on DVE (reciprocal_approx_fast on a [16,128] reshape
of the row sums) so ACT only ever runs Exp (one table load); AV staging and
V staging on DVE; V projection split into 4 t-groups interleaved with the
flash loops to keep the PE warm while ACT drains exps; bf16 at/wo; per-slab
weight DMAs + split DMA queues to cut the startup stall.
"""

import math
from contextlib import ExitStack

import numpy as np

import concourse.bass as bass
import concourse.tile as tile
from concourse import bacc, mybir

F32 = mybir.dt.float32
BF16 = mybir.dt.bfloat16

B, T, D, H, HD = 2, 2048, 1024, 16, 64
P = 128
KT = D // P            # 8 k-slabs for projections
NT = T // P            # 16 t/k tiles
QS = 512               # q-slab width for attention
NQS = T // QS          # 4 q-slabs
HPG = 4                # heads per core


def build_nc(num_devices=8):
    nc = bacc.Bacc("TRN2", target_bir_lowering=False, debug=False,
                   num_devices=num_devices)
    ext = dict(kind="ExternalInput")
    xT = nc.dram_tensor("xT", [D, T], BF16, **ext).ap()
    wq = nc.dram_tensor("wq", [D, 2 * P], BF16, **ext).ap()
    wk = nc.dram_tensor("wk", [D, 2 * P], BF16, **ext).ap()
    wv = nc.dram_tensor("wv", [D, 2 * P], BF16, **ext).ap()
    wo = nc.dram_tensor("wo", [2 * P, D], BF16, **ext).ap()
    csc = nc.dram_tensor("csc", [P, T], BF16, **ext).ap()
    ssc = nc.dram_tensor("ssc", [P, T], BF16, **ext).ap()
    bq2 = nc.dram_tensor("bq2", [P, 2], F32, **ext).ap()
    bk2 = nc.dram_tensor("bk2", [P, 2], F32, **ext).ap()
    bvr = nc.dram_tensor("bvr", [1, 2 * P], BF16, **ext).ap()
    m0 = nc.dram_tensor("m0", [P, P], BF16, **ext).ap()
    ones_in = nc.dram_tensor("ones_in", [P, P], BF16, **ext).ap()
    out = nc.dram_tensor("out", [T, D], F32, kind="ExternalOutput").ap()

    with tile.TileContext(nc) as tc:
        _body(tc, xT, wq, wk, wv, wo, csc, ssc, bq2, bk2, bvr, m0,
              ones_in, out)
    nc.compile()
    return nc


def _body(tc, xT, wq, wk, wv, wo, csc, ssc, bq2, bk2, bvr, m0,
          ones_in, out):
    nc = tc.nc
    Copy = mybir.ActivationFunctionType.Copy
    Exp = mybir.ActivationFunctionType.Exp
    Ident = mybir.ActivationFunctionType.Identity

    with ExitStack() as outer:
        consts = outer.enter_context(tc.tile_pool(name="consts", bufs=1))
        wpool = outer.enter_context(tc.tile_pool(name="w", bufs=1))
        xtp = outer.enter_context(tc.tile_pool(name="xt", bufs=1))
        qk = outer.enter_context(tc.tile_pool(name="qk", bufs=1))
        vp = outer.enter_context(tc.tile_pool(name="v", bufs=1))
        atp = outer.enter_context(tc.tile_pool(name="at", bufs=1))
        cscp = outer.enter_context(tc.tile_pool(name="cs", bufs=1))

        # DMA queues: gpsimd carries weights/consts + xts 4-7, sync carries
        # xts 0-3 (slab 0 column-chunked so the first matmul starts early),
        # vector carries csc/ssc (rope waits on them anyway). The scalar
        # queue issues nothing: ACT time is reserved for staging + exp.
        w_q = wpool.tile([P, KT, 2 * P], BF16, tag="wq")
        w_k = wpool.tile([P, KT, 2 * P], BF16, tag="wk")
        wv_s = wpool.tile([P, KT, 2 * P], BF16, tag="wv")
        wkr = wk.rearrange("(ko ki) m -> ki ko m", ki=P)
        nc.gpsimd.dma_start(w_k[:, 0, :], wkr[:, 0, :])
        nc.gpsimd.dma_start(w_k[:, 1:KT, :], wkr[:, 1:KT, :])

        xts = []
        xTr = xT.rearrange("(ko ki) t -> ki ko t", ki=P)
        for kt in range(KT):
            xc = xtp.tile([P, T], BF16, tag=f"xt{kt}")
            xts.append(xc)
        for n in range(4):
            cols = slice(n * 512, (n + 1) * 512)
            nc.sync.dma_start(xts[0][:, cols], xTr[:, 0, cols])
        for kt in (1, 2, 3):
            nc.sync.dma_start(xts[kt][:], xTr[:, kt, :])
        nc.gpsimd.dma_start(xts[4][:], xTr[:, 4, :])
        nc.gpsimd.dma_start(xts[5][:], xTr[:, 5, :])

        bq_s = consts.tile([P, 2], F32, tag="bq")
        bk_s = consts.tile([P, 2], F32, tag="bk")
        nc.gpsimd.dma_start(bk_s[:], bk2)
        nc.gpsimd.dma_start(bq_s[:], bq2)
        wqr = wq.rearrange("(ko ki) m -> ki ko m", ki=P)
        nc.gpsimd.dma_start(w_q[:], wqr)
        nc.gpsimd.dma_start(xts[6][:], xTr[:, 6, :])
        nc.gpsimd.dma_start(xts[7][:], xTr[:, 7, :])
        wvr = wv.rearrange("(ko ki) m -> ki ko m", ki=P)
        nc.gpsimd.dma_start(wv_s[:], wvr)
        bv_s = consts.tile([1, 2 * P], BF16, tag="bv")
        nc.gpsimd.dma_start(bv_s[:], bvr)
        m0_s = consts.tile([P, P], BF16, tag="m0")
        nc.gpsimd.dma_start(m0_s[:], m0)
        ones_s = consts.tile([1, P], BF16, tag="ones")
        nc.gpsimd.dma_start(ones_s[:], ones_in[0:1, :])
        wop = outer.enter_context(tc.tile_pool(name="wo", bufs=1))
        wo_s = wop.tile([P, 2, D], BF16)
        nc.gpsimd.dma_start(
            wo_s[:], wo.rearrange("(ko ki) m -> ki ko m", ki=P))

        # V sbuf store (per kt-tile, per head, HD cols + ones col for sums)
        v_s = vp.tile([P, NT, HPG, HD + 1], BF16)
        nc.vector.memset(v_s[:, :, :, HD:HD + 1], 1.0)
        csc_s = cscp.tile([P, T], BF16, tag="c")
        nc.sync.dma_start(csc_s[:], csc)
        ssc_s = cscp.tile([P, T], BF16, tag="s")
        nc.sync.dma_start(ssc_s[:], ssc)

        at0 = atp.tile([P, T], BF16, tag="at0")
        at1 = atp.tile([P, T], BF16, tag="at1")
        at_tiles = (at0, at1)

        qc, kc = [], []

        def emit_vgroup(pool, tag, g, stage_on_act=False):
            # V projection for t-tiles 4g..4g+3 -> v_s (+bias via ones row).
            # stage_on_act: psum->sbuf staging on ScalarE (used in phase A
            # where ACT is idle and the DVE rope/merge chain is critical —
            # DVE-tail staging would also gate the PSUM pool handover).
            tps = (2 * g, 2 * g + 1)
            vss = {}
            for tp in tps:
                vss[tp] = pool.tile([P, 2, 2 * P], F32, tag=tag,
                                    name=f"vps{tp}")
            for kt in range(KT):
                for tp in tps:
                    for half in range(2):
                        t = 2 * tp + half
                        nc.tensor.matmul(
                            vss[tp][:, half, :],
                            xts[kt][:, t * P:(t + 1) * P],
                            wv_s[:, kt, :],
                            start=(kt == 0 and half == 0), stop=False,
                        )
            for tp in tps:
                for half in range(2):
                    nc.tensor.matmul(
                        vss[tp][:, half, :], ones_s[:], bv_s[:],
                        start=False, stop=(half == 1),
                    )
                if stage_on_act:
                    nc.scalar.activation(
                        out=v_s[:, 2 * tp:2 * tp + 2, :, 0:HD],
                        in_=vss[tp].rearrange("p c (h d) -> p c h d", h=HPG),
                        func=Copy, scale=1.0,
                    )
                else:
                    nc.vector.tensor_copy(
                        v_s[:, 2 * tp:2 * tp + 2, :, 0:HD],
                        vss[tp].rearrange("p c (h d) -> p c h d", h=HPG),
                    )

        # ============ phase A: Q/K projections + RoPE, V groups 0-2 ========
        with ExitStack() as pha:
            rtmp = pha.enter_context(tc.tile_pool(name="rtmp", bufs=4))
            ps_proj = pha.enter_context(
                tc.tile_pool(name="psp", bufs=8, space="PSUM"))

            raw = {}
            # K first: its stage/rope/merge chain runs on ACT/DVE while the
            # PE grinds the Q projection. K in ascending column halves (the
            # first flash consumes low k first); Q in DESCENDING column
            # quarters (flash(qs=3) reads q columns 1536+ first).
            for name, w_s, b_s, chunks in (
                    ("k", w_k, bk_s, [0, 1]),
                    ("q", w_q, bq_s, [3, 2, 1, 0])):
                nch = len(chunks)
                cw = T // nch
                pss = []
                for _i in range(8):
                    pst = ps_proj.tile([P, 512], F32, tag="ps", name=f"ps{_i}")
                    pss.append(pst)
                for kt in range(KT):
                    for m in range(2):
                        for n in range(T // 512):
                            nc.tensor.matmul(
                                pss[m * 4 + n][:],
                                w_s[:, kt, m * P:(m + 1) * P],
                                xts[kt][:, n * 512:(n + 1) * 512],
                                start=(kt == 0), stop=(kt == KT - 1),
                            )
                for m in range(2):
                    rt = qk.tile([P, T], BF16, tag=f"raw{name}{m}")
                    raw[(name, m)] = rt
                nst = T // 512
                for n in (range(nst) if name == "k" else range(nst - 1, -1, -1)):
                    for m in range(2):
                        nc.scalar.activation(
                            out=raw[(name, m)][:, n * 512:(n + 1) * 512],
                            in_=pss[m * 4 + n][:],
                            func=Ident, bias=b_s[:, m:m + 1], scale=1.0,
                        )

                # RoPE + merge right after each projection's staging.
                # Layout per merged tile j: heads (2j, 2j+1); head h occupies
                # partitions 64*(h%2)..+63 as [x1(32); x2(32)].
                x1, x2 = raw[(name, 0)], raw[(name, 1)]
                tgts = []
                for j in range(2):
                    tgt = qk.tile([P, T], BF16, tag=f"c{name}{j}")
                    tgts.append(tgt)
                (qc if name == "q" else kc).extend(tgts)
                for ch in chunks:
                    cols = slice(ch * cw, (ch + 1) * cw)
                    t1 = rtmp.tile([P, cw], BF16, tag="rt", name="t1")
                    nc.vector.tensor_mul(t1[:], x1[:, cols], csc_s[:, cols])
                    t2 = rtmp.tile([P, cw], BF16, tag="rt", name="t2")
                    nc.vector.tensor_mul(t2[:], x2[:, cols], ssc_s[:, cols])
                    t3 = rtmp.tile([P, cw], BF16, tag="rt", name="t3")
                    nc.vector.tensor_mul(t3[:], x1[:, cols], ssc_s[:, cols])
                    t4 = rtmp.tile([P, cw], BF16, tag="rt", name="t4")
                    nc.vector.tensor_mul(t4[:], x2[:, cols], csc_s[:, cols])
                    y1 = qk.tile([P, cw], BF16, tag=f"y{name}0", name="y1")
                    nc.vector.tensor_sub(y1[:], t1[:], t2[:])
                    y2 = qk.tile([P, cw], BF16, tag=f"y{name}1", name="y2")
                    nc.vector.tensor_add(y2[:], t3[:], t4[:])
                    # merge halves into head-interleaved tiles (DVE copies;
                    # ScalarE copies run at 1x and stall the ACT queue)
                    for j in range(2):
                        tgt = tgts[j]
                        for i in range(2):
                            h = 2 * j + i
                            hs = slice(32 * h, 32 * h + 32)
                            nc.vector.tensor_copy(
                                tgt[64 * i:64 * i + 32, cols], y1[hs, :])
                            nc.vector.tensor_copy(
                                tgt[64 * i + 32:64 * i + 64, cols], y2[hs, :])

            emit_vgroup(ps_proj, "ps", 0, stage_on_act=True)
            emit_vgroup(ps_proj, "ps", 1, stage_on_act=True)

        # ================= phase B: attention =================
        with ExitStack() as phb:
            expp = phb.enter_context(tc.tile_pool(name="exp", bufs=6))
            avsp = phb.enter_context(tc.tile_pool(name="avs", bufs=2))
            rrp = phb.enter_context(tc.tile_pool(name="rr", bufs=2))
            s16p = phb.enter_context(tc.tile_pool(name="s16", bufs=2))
            ps_sc = phb.enter_context(
                tc.tile_pool(name="pssc", bufs=2, space="PSUM"))
            ps_av = phb.enter_context(
                tc.tile_pool(name="psav", bufs=1, space="PSUM"))
            drp = phb.enter_context(
                tc.tile_pool(name="dr", bufs=2, space="DRAM"))
            outb = phb.enter_context(tc.tile_pool(name="outb", bufs=3))

            staged = {}

            def emit_rchain(qs, sums_src):
                # r = 1/sums via DVE reciprocal_approx_fast on a [16,128]
                # reshape (DRAM hop), broadcast back over the HD partitions
                # as bf16.
                avs, _ = staged[qs]
                d_sums = drp.tile([HPG * QS], F32, tag="ds", name="d_sums")
                nc.sync.dma_start(d_sums[None, :],
                                  sums_src.rearrange("o h q -> o (h q)"))
                s16 = s16p.tile([16, P], F32, tag="s16", name="s16")
                nc.sync.dma_start(s16[:],
                                  d_sums.rearrange("(p c) -> p c", p=16))
                r16 = s16p.tile([16, P], F32, tag="r16", name="r16")
                nc.vector.reciprocal_approx_fast(out=r16[:], in_=s16[:])
                r16b = s16p.tile([16, P], BF16, tag="r16b", name="r16b")
                nc.vector.tensor_copy(r16b[:], r16[:])
                d_r = drp.tile([HPG * QS], BF16, tag="dr", name="d_r")
                nc.sync.dma_start(d_r.rearrange("(p c) -> p c", p=16),
                                  r16b[:])
                rr = rrp.tile([HD, HPG, QS], BF16, tag="rr", name="rr")
                nc.sync.dma_start(
                    rr[:], d_r[None, :].broadcast_to([HD, HPG * QS])
                    .rearrange("p (h q) -> p h q", h=HPG))
                staged[qs] = (avs, rr)

            def emit_normalize(qs, fine=False):
                avs, rr = staged.pop(qs)
                if not fine:
                    for tg in range(2):
                        for i in range(2):
                            h = 2 * tg + i
                            nc.vector.tensor_mul(
                                at_tiles[tg][i * HD:(i + 1) * HD,
                                             qs * QS:(qs + 1) * QS],
                                avs[0:HD, h, :], rr[:, h, :],
                            )
                else:
                    # per-qt columns so outproj can chase the muls
                    for qt in range(4 * qs, 4 * qs + 4):
                        c0 = qt * P - qs * QS
                        for tg in range(2):
                            for i in range(2):
                                h = 2 * tg + i
                                nc.vector.tensor_mul(
                                    at_tiles[tg][i * HD:(i + 1) * HD,
                                                 qt * P:(qt + 1) * P],
                                    avs[0:HD, h, c0:c0 + P],
                                    rr[:, h, c0:c0 + P],
                                )
                        emit_outproj_qt(qt, ring=nc.scalar)

            def emit_outproj_qt(qt, ring=None):
                ps = ps_sc.tile([P, 2, QS], F32, tag="sc", name="ops")
                ob = outb.tile([P, D], F32, tag="ob", name="ob")
                for nb in range(2):
                    for ktg in range(2):
                        nc.tensor.matmul(
                            ps[:, nb, :],
                            at_tiles[ktg][:, qt * P:(qt + 1) * P],
                            wo_s[:, ktg, nb * 512:(nb + 1) * 512],
                            start=(ktg == 0), stop=(ktg == 1),
                        )
                    nc.vector.tensor_copy(
                        out=ob[:, nb * 512:(nb + 1) * 512],
                        in_=ps[:, nb, :])
                if ring is None:
                    ring = nc.gpsimd
                ring.dma_start(out[qt * P:(qt + 1) * P, :], ob[:])

            def emit_outproj(qs):
                for qt in range(4 * qs, 4 * qs + 4):
                    emit_outproj_qt(qt)

            # Descending q-slab order: the biggest exp workload starts first
            # (keeps ACT saturated while later PE work drains), the kernel
            # ends on the smallest slab, and out-DMAs spread evenly. V
            # groups 1-3 are interleaved into the qs=3 kt-loop right before
            # the AV matmuls that need them, filling PE while ACT chews the
            # early exp backlog.
            QORDER = [3, 2, 1, 0]
            for qidx, qs in enumerate(QORDER):
                av = ps_av.tile([HD + 1, HPG, QS], F32, tag="av", name="av")
                n_kt = 4 * qs + 4
                for kt in range(n_kt):
                    if qidx == 0 and kt in (4, 8):
                        emit_vgroup(ps_sc, "sc", 1 + kt // 4)
                    qoff = max(0, kt * P - qs * QS)
                    q0 = qs * QS + qoff
                    qext = QS - qoff
                    diag = kt * P >= qs * QS
                    for pair in range(2):
                        sc = ps_sc.tile([P, 2, QS], F32, tag="sc", name="sc")
                        for i in range(2):
                            nc.tensor.matmul(
                                sc[:, i, qoff:QS],
                                kc[pair][64 * i:64 * i + 64,
                                         kt * P:(kt + 1) * P],
                                qc[pair][64 * i:64 * i + 64, q0:q0 + qext],
                                start=True, stop=True,
                                tile_position=(64 * i, 0),
                            )
                        ex = expp.tile([P, 2, QS], BF16, tag="ex", name="ex")
                        nc.scalar.activation(
                            out=ex[:, :, qoff:QS], in_=sc[:, :, qoff:QS],
                            func=Exp, scale=1.0,
                        )
                        if diag:
                            # diagonal k-tile: zero strictly-upper corner
                            nc.vector.tensor_mul(
                                ex[:, :, qoff:qoff + P],
                                ex[:, :, qoff:qoff + P],
                                m0_s[:, None, :].broadcast_to([P, 2, P]),
                            )
                        for i in range(2):
                            h = 2 * pair + i
                            nc.tensor.matmul(
                                av[:, h, qoff:QS],
                                v_s[:, kt, h, :],
                                ex[:, i, qoff:QS],
                                start=(kt == 0), stop=(kt == n_kt - 1),
                            )
                # stage AV psum (+ sums row) to SBUF, then the r chain. For
                # the last slab the sums row goes through ACT (idle in the
                # tail) so the r chain doesn't wait on the big DVE copy.
                avs = avsp.tile([HD + 1, HPG, QS], F32, tag="avs", name="avs")
                last = qidx == len(QORDER) - 1
                if last:
                    sums_sb = s16p.tile([1, HPG, QS], F32, tag="sm",
                                        name="sums_sb")
                    nc.scalar.activation(out=sums_sb[:],
                                         in_=av[HD:HD + 1, :, :],
                                         func=Copy, scale=1.0)
                    sums_src = sums_sb[:]
                else:
                    sums_src = avs[HD:HD + 1, :, :]
                nc.vector.tensor_copy(avs[:], av[:])
                staged[qs] = (avs, None)
                emit_rchain(qs, sums_src)

                if qidx > 0:
                    emit_normalize(QORDER[qidx - 1])
                    emit_outproj(QORDER[qidx - 1])

            emit_normalize(QORDER[-1], fine=True)


# ---------------- host-side prep ----------------

def _perm(g):
    perm = []
    for half in range(2):
        for hh in range(HPG):
            for i in range(32):
                perm.append(256 * g + 64 * hh + 2 * i + half)
    return np.array(perm)


def host_inputs(inputs, c):
    b, g = c // 4, c % 4
    x, cos, sin = inputs["x"], inputs["cos"], inputs["sin"]
    Wq, bq, Wk, bk = inputs["Wq"], inputs["bq"], inputs["Wk"], inputs["bk"]
    Wv, bv, Wo = inputs["Wv"], inputs["bv"], inputs["Wo"]
    perm = _perm(g)
    s = math.sqrt(1.0 / math.sqrt(HD))
    cosT = np.ascontiguousarray(cos[0, 0].T) * s    # [32, T]
    sinT = np.ascontiguousarray(sin[0, 0].T) * s
    f32 = np.float32
    import ml_dtypes
    bf16 = ml_dtypes.bfloat16
    return {
        "xT": np.ascontiguousarray(x[b].T).astype(bf16),
        "wq": np.ascontiguousarray(Wq[perm, :].T).astype(bf16),
        "wk": np.ascontiguousarray(Wk[perm, :].T).astype(bf16),
        "wv": np.ascontiguousarray(Wv[256 * g:256 * (g + 1), :].T).astype(bf16),
        "wo": np.ascontiguousarray(Wo[:, 256 * g:256 * (g + 1)].T).astype(bf16),
        "csc": np.ascontiguousarray(np.tile(cosT, (4, 1))).astype(bf16),
        "ssc": np.ascontiguousarray(np.tile(sinT, (4, 1))).astype(bf16),
        "bq2": np.ascontiguousarray(bq[perm].reshape(2, P).T).astype(f32),
        "bk2": np.ascontiguousarray(bk[perm].reshape(2, P).T).astype(f32),
        "bvr": np.ascontiguousarray(
            bv[256 * g:256 * (g + 1)].reshape(1, 2 * P)).astype(bf16),
        "m0": np.ascontiguousarray(
            (np.arange(P)[None, :] >= np.arange(P)[:, None])).astype(bf16),
        "ones_in": np.ones((P, P), bf16),
    }


def host_gather(results, bo):
    out = np.zeros((B, T, D), np.float32)
    for c in range(8):
        out[c // 4] += results[c]["out"]
    out += bo[None, None, :]
    return out


_NC_CACHE = {}


def _get_nc():
    if "nc" not in _NC_CACHE:
        _NC_CACHE["nc"] = build_nc(num_devices=8)
    return _NC_CACHE["nc"]


def kernel(**inputs):
    inputs = {k: np.asarray(v) for k, v in inputs.items()}
    nc = _get_nc()
    from concourse.bass_utils import run_bass_kernel_spmd
    in_maps = [host_inputs(inputs, c) for c in range(8)]
    res = run_bass_kernel_spmd(nc, in_maps, core_ids=list(range(8)))
    return host_gather(res.results, inputs["bo"].astype(np.float32))
